# revision 1
# baseline (speedup 1.0000x reference)
"""Trainium2 Bass kernel for MinibatchDiscrimination (v2).

Reference computation (fp32):
    m = (x @ W.T + b).reshape(nb, 64, 16)            # nb=512
    d[i,j,B] = sum_c |m[i,B,c] - m[j,B,c]|
    o[i,B]   = sum_j exp(-d[i,j,B])
    out      = concat(x, o, axis=1)                   # (512, 1088)

Strategy (8 cores, symmetric-pair sharding): core c owns output rows
R_c = [64c, 64c+64).  x is row-rotated per core so R_c is local rows
0..63; the core computes exp(-d) only for the 320-column local window
j in [0, 320) (= R_c..R_{c+4}) and exposes, in addition to the row sums
over its window, per-column sums over local columns [64, 256).  By
symmetry of d those column sums are exactly the row-sum contributions
the owners of R_{c+1},R_{c+2},R_{c+3} are missing; the host adds the
partials (pure numpy, negligible).

On-device pipeline per core:
  mT[t] = W @ x^T + b  as 8 fp16 tiles [128 (B,c), 320 j]   (PE, fp32r
          matmuls: full rate + ~fp32 precision; bias-add evac on DVE,
          fp32 columns 0..64 kept separately for scalar operands)
  Feature tiles 0..5 use the min path (DVE/Pool tensor_scalar_min,
  fp16 in/out = DVE 4x mode), tiles 6..7 the abs path (ACT Abs with
  per-partition bias).  d-loop runs tile-outer over groups of GROUP
  PSUM pair-banks so consecutive matmuls share their stationary; a
  post-pass deletes the redundant ldweights (PE seq was the old
  bottleneck) -- invalidated across self-loading (fp32r) matmuls.
  With S = sum_c m over min tiles (psq matmul, fp16 exact in PSUM):
    exp(-d) = exp(psd - S_i) * exp(-S_j)
    E  = Exp(psd + bias=-S_i)          (ACT, fp32)
    Escr = E*Q2 -> fp16, accum_out oacc (DVE stt)
    colsum: ones16^T @ Escr[:, 64:256] accumulated over all 32 pair
    tiles into one PSUM bank (PE; ~3us) -> cpart [64, 192]
Host assembles: out = concat(x, rowsums + scattered colsums, axis=1).

reps>1 unrolls the whole body N times in one program (same tiles, so
the Tile dep-tracker serializes the repetitions) -- used only to time
the kernel with the fixed per-dispatch overhead amortized away.

The container's walrus rejects >1 sync wait per instruction; the
_split_multi_waits pass legalizes (hoists extras onto NoOps)."""

import os
import sys
import numpy as np

if "/opt/trn_rl_repo" not in sys.path:
    sys.path.insert(0, "/opt/trn_rl_repo")

NB = 512          # batch rows
NIN = 1024        # n_in
NBF = 64          # n_B
NCD = 16          # n_C
FOUT = NBF * NCD  # 1024 projection features
NCORES = 8
IB = NB // NCORES  # 64 output rows per core
WIN = 320          # local j-window per core (512 = no symmetry)
NMIN = 7           # tiles 0..NMIN-1 -> min path; rest -> ACT abs path
NPAIR = IB // 2    # 32 psd pair tiles
GROUP = 5          # pairs per psd PSUM group
CSLO, CSHI = 64, 256  # local column range exported as colsum partials

_CACHE = {}


def _build_program(reps=1):
    import concourse.bass as bass
    import concourse.tile as tile
    from concourse import mybir
    from contextlib import ExitStack

    f32 = mybir.dt.float32
    f32r = mybir.dt.float32r
    f16 = mybir.dt.float16
    Alu = mybir.AluOpType
    Act = mybir.ActivationFunctionType

    use_sym = WIN != NB

    nc = bass.Bass()
    xTr_d = nc.declare_dram_parameter("xTr", [NIN, WIN], f16, isOutput=False)
    wTr_d = nc.declare_dram_parameter("wTr", [NIN, FOUT], f16, isOutput=False)
    b_d = nc.declare_dram_parameter("b", [FOUT], f32, isOutput=False)
    ind16_d = nc.declare_dram_parameter("ind16", [8 * 128, NBF], f16, isOutput=False)
    ones16_d = nc.declare_dram_parameter("ones16", [128, NBF], f16, isOutput=False)
    o_d = nc.declare_dram_parameter("o", [128, NPAIR], f32, isOutput=True)
    cp_d = None
    if use_sym:
        cp_d = nc.declare_dram_parameter("cpart", [NBF, CSHI - CSLO], f32, isOutput=True)

    with tile.TileContext(nc) as tc, ExitStack() as ctx:
        singles = ctx.enter_context(tc.tile_pool(name="singles", bufs=1))
        wstream = ctx.enter_context(tc.tile_pool(name="wstream", bufs=1))
        scr16 = ctx.enter_context(tc.tile_pool(name="scr16", bufs=36))
        scrA = ctx.enter_context(tc.tile_pool(name="scrA", bufs=8))
        epool = ctx.enter_context(tc.tile_pool(name="epool", bufs=8))
        psA = ctx.enter_context(tc.tile_pool(name="psA", bufs=2, space="PSUM"))
        psQ = ctx.enter_context(tc.tile_pool(name="psQ", bufs=1, space="PSUM"))
        psB = ctx.enter_context(tc.tile_pool(name="psB", bufs=GROUP, space="PSUM"))

        dma = nc.default_dma_engine

        # ---- persistent loads -------------------------------------------
        # first W slab goes out before the x tiles so the PE's first
        # ldweights isn't gated on the whole x transfer
        wsl0_first = wstream.tile([128, FOUT], f16, name="wsl0", tag="wsl0")
        dma.dma_start(out=wsl0_first, in_=wTr_d[0:128, :])
        xr = []
        for k in range(8):
            t_ = singles.tile([128, WIN], f16, name=f"xr{k}", tag=f"xr{k}")
            dma.dma_start(out=t_, in_=xTr_d[128 * k : 128 * (k + 1), :])
            xr.append(t_)
        b_sb = singles.tile([128, 8], f32, name="b_sb", tag="b_sb")
        ind_sb = [singles.tile([128, NBF], f16, name=f"ind{t}", tag=f"ind{t}")
                  for t in range(8)]
        ones_sb = singles.tile([128, NBF], f16, name="ones16", tag="ones16")
        aux_loaded = [False]

        def load_aux():
            # issued after the first wslab stream so these 10 small DMAs
            # don't delay the critical W/x loads on the serial HWDGE
            dma.dma_start(out=b_sb, in_=b_d.rearrange("(t p) -> p t", p=128))
            for t in range(8):
                dma.dma_start(out=ind_sb[t], in_=ind16_d[128 * t : 128 * (t + 1), :])
            dma.dma_start(out=ones_sb, in_=ones16_d[:, :])
            aux_loaded[0] = True

        # persistent compute tiles (rewritten each rep)
        mt16 = [singles.tile([128, WIN], f16, name=f"mt{t}", tag=f"mt{t}")
                for t in range(8)]
        mc32 = [singles.tile([128, IB], f32, name=f"mc{t}", tag=f"mc{t}")
                for t in range(8)]
        negS2 = singles.tile([128, NPAIR], f32, name="negS2", tag="negS2")
        Q2 = singles.tile([128, WIN], f32, name="Q2", tag="Q2")
        oacc = singles.tile([128, NPAIR], f32, name="oacc", tag="oacc")
        cp_sb = None
        if use_sym:
            cp_sb = singles.tile([NBF, CSHI - CSLO], f32, name="cp", tag="cp")

        def min_engine(t, i):
            # balance ~12 of each group's 56 min ops onto Pool (gpsimd)
            if t == NMIN - 1:
                return nc.gpsimd
            if t == NMIN - 2 and i % 2 == 0:
                return nc.gpsimd
            return nc.vector

        def one_rep():
            # ---- mT = W @ x^T + b (fp32r) -------------------------------
            wslab = []
            for kb in range(8):
                if kb == 0 and not aux_loaded[0]:
                    wslab.append(wsl0_first)
                    continue
                sl = wstream.tile([128, FOUT], f16, name=f"wsl{kb}", tag=f"wsl{kb}")
                dma.dma_start(out=sl, in_=wTr_d[128 * kb : 128 * (kb + 1), :])
                wslab.append(sl)
            if not aux_loaded[0]:
                load_aux()
            for t in range(8):
                ps = psA.tile([128, WIN], f32, name="mps", tag="mps")
                for kb in range(8):
                    nc.tensor.matmul(
                        ps, lhsT=wslab[kb][:, 128 * t : 128 * (t + 1)],
                        rhs=xr[kb], start=(kb == 0), stop=(kb == 7)
                    )
                nc.vector.tensor_scalar_add(mt16[t], ps, b_sb[:, t : t + 1])
                nc.scalar.activation(
                    out=mc32[t], in_=ps[:, 0:IB], func=Act.Identity,
                    bias=b_sb[:, t : t + 1], scale=1.0,
                )

            # ---- psq = 2*S over min tiles; negS2, Q2 --------------------
            psq = psQ.tile([NBF, WIN], f32, name="psq", tag="psq")
            for t in range(NMIN):
                nc.tensor.matmul(
                    psq, lhsT=ind_sb[t], rhs=mt16[t],
                    start=(t == 0), stop=(t == NMIN - 1),
                )
            psq_pairs = psq[:, 0:IB].rearrange("b (p two) -> b two p", two=2)
            nc.scalar.activation(
                out=negS2[0:NBF, :], in_=psq_pairs[:, 0, :],
                func=Act.Copy, bias=0.0, scale=-0.5,
            )
            nc.scalar.activation(
                out=negS2[NBF:128, :], in_=psq_pairs[:, 1, :],
                func=Act.Copy, bias=0.0, scale=-0.5,
            )
            nc.scalar.activation(out=Q2[0:NBF, :], in_=psq, func=Act.Exp,
                                 bias=0.0, scale=-0.5)
            nc.scalar.activation(out=Q2[NBF:128, :], in_=psq, func=Act.Exp,
                                 bias=0.0, scale=-0.5)

            cacc = None
            if use_sym:
                cacc = psQ.tile([NBF, CSHI - CSLO], f32, name="cacc", tag="psq")

            # ---- pairwise loop: tile-outer over groups ------------------
            # ragged group first: it overlaps the projection ramp and
            # leaves a full-size group to hide the epilogue tail
            rag = NPAIR % GROUP
            starts = ([0] if rag else []) + list(range(rag, NPAIR, GROUP))
            for g0 in starts:
                pairs = list(range(g0, min(g0 + (rag if (rag and g0 == 0) else GROUP), NPAIR)))
                psd = {}
                for p in pairs:
                    psd[p] = psB.tile([128, WIN], f32, name="psd", tag="psd")
                E_ORDER = (0, 7, 6, 1, 5, 2, 3, 4)
                for n_e, e in enumerate(E_ORDER):
                    for h in range(2):
                        for p in pairs:
                            i = 2 * p + h
                            out_ap = psd[p][NBF * h : NBF * (h + 1), :]
                            if e < NMIN:
                                mn = scr16.tile([128, WIN], f16, name="mn", tag="mn")
                                min_engine(e, i).tensor_scalar_min(
                                    mn, mt16[e], mc32[e][:, i : i + 1]
                                )
                            else:
                                mn = scrA.tile([128, WIN], f16, name="ab", tag="ab")
                                nc.scalar.activation(
                                    out=mn, in_=mt16[e], func=Act.Abs,
                                    bias=mc32[e][:, i : i + 1], scale=-1.0,
                                )
                            nc.tensor.matmul(
                                out_ap, lhsT=ind_sb[e], rhs=mn,
                                start=(n_e == 0), stop=(n_e == 7),
                            )
                escr_tiles = {}
                for p in pairs:
                    E = epool.tile([128, WIN], f32, name="E", tag="E")
                    nc.scalar.activation(
                        out=E, in_=psd[p], func=Act.Exp,
                        bias=negS2[:, p : p + 1], scale=1.0,
                    )
                    Escr = epool.tile([128, WIN], f16, name="Escr", tag="Escr")
                    nc.vector.scalar_tensor_tensor(
                        out=Escr, in0=E, scalar=1.0, in1=Q2,
                        op0=Alu.mult, op1=Alu.mult,
                        accum_out=oacc[:, p : p + 1],
                    )
                    escr_tiles[p] = Escr
                if use_sym:
                    # batched so the ones16 stationary loads once per group
                    for p in pairs:
                        nc.tensor.matmul(
                            cacc, lhsT=ones_sb, rhs=escr_tiles[p][:, CSLO:CSHI],
                            start=(p == 0), stop=(p == NPAIR - 1),
                            skip_group_check=True,
                        )

            dma.dma_start(out=o_d[:, :], in_=oacc)
            if use_sym:
                nc.scalar.activation(out=cp_sb, in_=cacc, func=Act.Copy,
                                     bias=0.0, scale=1.0)
                dma.dma_start(out=cp_d[:, :], in_=cp_sb)

        for _ in range(reps):
            one_rep()

    _dedup_ldweights(nc)
    _split_multi_waits(nc, mybir)
    return nc


def _dedup_ldweights(nc):
    """Drop PE InstLdweights whose weights AP + mode matches the previous
    load and which carry no sync (weights stay resident in the PE array).
    Self-loading matmuls (fp32/fp32r) clobber resident weights and
    invalidate the tracking."""
    f = nc.m.functions[0]
    n = 0
    for blk in f.blocks:
        last_key = None
        keep = []
        for inst in blk.instructions:
            if str(inst.engine) == "EngineType.PE":
                tname = type(inst).__name__
                if tname == "InstLdweights":
                    si = inst.sync_info
                    clean = not si or (not si.on_wait and not si.on_update)
                    key = (
                        str(inst.ins[0]),
                        str(inst.perf_mode),
                        str(inst.is_transpose),
                        str(inst.tile_position),
                    )
                    if clean and key == last_key:
                        n += 1
                        continue
                    last_key = key
                elif tname == "InstMatmult":
                    if inst.ldweights is not False:
                        last_key = None
            keep.append(inst)
        blk.instructions[:] = keep
    return n


def _split_multi_waits(nc, mybir):
    """Walrus here rejects >1 sync wait per instruction; hoist extras onto
    single-wait NoOps just before, on the same engine queue."""
    f = nc.m.functions[0]
    n_split = 0
    for blk in f.blocks:
        idx = 0
        while idx < len(blk.instructions):
            inst = blk.instructions[idx]
            si = inst.sync_info
            waits = list(si.on_wait) if si is not None and si.on_wait else []
            if len(waits) > 1:
                bysem = {}
                for w in waits:
                    k = w.id
                    if k not in bysem or (w.wait_value or 0) > (
                        bysem[k].wait_value or 0
                    ):
                        bysem[k] = w
                waits = list(bysem.values())
                for w in waits[:-1]:
                    nop = mybir.InstNoOp(
                        name=nc.get_next_instruction_name(), ins=[], outs=[]
                    )
                    nop.engine = inst.engine
                    nop.sync_info = mybir.SyncInfo(on_wait=[w], on_update=[])
                    blk.instructions.insert(idx, nop)
                    idx += 1
                    n_split += 1
                si.on_wait = [waits[-1]]
            idx += 1
    return n_split


def _get_program(reps=1):
    key = f"nc{reps}"
    if key not in _CACHE:
        _CACHE[key] = _build_program(reps)
    return _CACHE[key]


def _round_f32r(a):
    return (np.ascontiguousarray(a, dtype=np.float32).view(np.uint32)
            & np.uint32(0xFFFFE000)).view(np.float32)


def _make_indicators():
    ind16 = np.zeros((8 * 128, NBF), dtype=np.float16)
    r = np.arange(8 * 128)
    ind16[r, (r // 128) * 8 + (r % 128) // NCD] = np.where(
        r // 128 < NMIN, 2.0, -1.0
    )
    ones16 = np.zeros((128, NBF), dtype=np.float16)
    bb = np.arange(NBF)
    ones16[bb, bb] = 1.0
    ones16[NBF + bb, bb] = 1.0
    return ind16, ones16


def make_in_maps(x, W, b):
    x = np.ascontiguousarray(x, dtype=np.float32)
    W = np.ascontiguousarray(W, dtype=np.float32)
    b = np.ascontiguousarray(b, dtype=np.float32)
    ind16, ones16 = _make_indicators()
    wTr = np.ascontiguousarray(W.T).astype(np.float16)
    in_maps = []
    for c in range(NCORES):
        xr = np.roll(x, -IB * c, axis=0)
        xTr = np.ascontiguousarray(xr.T[:, :WIN]).astype(np.float16)
        in_maps.append({
            "xTr": xTr, "wTr": wTr, "b": b,
            "ind16": ind16, "ones16": ones16,
        })
    return in_maps


def kernel(x, W, b):
    from concourse.bass_utils import run_bass_kernel_spmd

    x = np.ascontiguousarray(x, dtype=np.float32)
    nc = _get_program()
    in_maps = make_in_maps(x, W, b)

    res = run_bass_kernel_spmd(nc, in_maps, list(range(NCORES)), trace=False)
    _CACHE["last_results"] = res

    o_full = np.zeros((NB, NBF), dtype=np.float64)
    for c in range(NCORES):
        oc = np.asarray(res.results[c]["o"], dtype=np.float64)
        o_core = np.empty((IB, NBF))
        o_core[0::2, :] = oc[0:NBF, :].T
        o_core[1::2, :] = oc[NBF:128, :].T
        o_full[IB * c : IB * (c + 1), :] += o_core
        if WIN != NB:
            cp = np.asarray(res.results[c]["cpart"], dtype=np.float64)
            rows = (IB * c + np.arange(CSLO, CSHI)) % NB
            o_full[rows, :] += cp.T
    return np.concatenate([x, o_full.astype(np.float32)], axis=1)



# revision 9
# speedup vs baseline: 1.0598x; 1.0598x over previous
"""Trainium2 Bass kernel for MinibatchDiscrimination (v3).

Reference computation (fp32):
    m = (x @ W.T + b).reshape(nb, 64, 16)            # nb=512
    d[i,j,B] = sum_c |m[i,B,c] - m[j,B,c]|
    o[i,B]   = sum_j exp(-d[i,j,B])
    out      = concat(x, o, axis=1)                   # (512, 1088)

Strategy (8 cores, 32-ring symmetric decomposition): core c owns output
rows R_c = [64c, 64c+64), x row-rotated per core.  Rows are split into
4 sub-blocks of 16; the 512 columns form a ring of 32 16-blocks.  A row
in 16-block r computes exp(-d) over the 272-column window of blocks
r..r+16 (local cols [16s, 16s+272) for sub-block s).  Row sums cover
distances 0..+16; per-column sums over window cols [16,256) (distances
+1..+15) are exported and added host-side to the owners of those rows
(by symmetry of d).  Distance-16 blocks are computed by both endpoint
blocks, each feeding only its own row sums -- no double count.

On-device pipeline per core:
  mT = W @ x^T + b as tiles [128 (B,c), 320 j]: fp8 DoubleRow matmuls
      (x, 64*W in fp8e4m3; PSUM scaled by 1/64 on evacuation).  Tiles
      0..5 evacuate to fp16 (DVE), tiles 6..7 to fp8 (ACT) for the
      fp8 pairwise path.
  Per row i: tiles 0..5 min-path on DVE (fp16, 4x mode); tile 6
      min-path on Pool (fp8); tile 7 abs-path on ACT (fp8).  The two
      fp8 results land in the halves of one [128,2,272] tile consumed
      by a single DoubleRow matmul (0.5 cyc/row); fp16 tiles use 6
      plain matmuls.  A per-pair replicate matmul folds -S_j into the
      same PSUM accumulation, so ACT's Exp (bias=-S_i) directly yields
      exp(-d) in fp16 with accum_out giving the row sums for free.
  colsum: ones16^T @ E[:,16:256] accumulated per sub-block into
      partition-disjoint PSUM regions; evacuated once.

Host assembles: out = concat(x, rowsums + scattered colsums, axis=1).

reps>1 unrolls the body for marginal (steady-state) timing.
The container's walrus rejects >1 sync wait per instruction; the
_split_multi_waits pass legalizes (hoists extras onto NoOps)."""

import os
import sys
import numpy as np

if "/opt/trn_rl_repo" not in sys.path:
    sys.path.insert(0, "/opt/trn_rl_repo")

NB = 512          # batch rows
NIN = 1024        # n_in
NBF = 64          # n_B
NCD = 16          # n_C
FOUT = NBF * NCD  # 1024 projection features
NCORES = 8
IB = NB // NCORES  # 64 output rows per core
TILEW = 320        # mt tile width (union of sub-block windows)
WIN = 272          # per-row j-window (17 x 16-blocks)
SUBW = 16          # sub-block row granularity
NSUB = IB // SUBW  # 4 sub-blocks per core
NPAIR = IB // 2    # 32 psd pair tiles
PPS = NPAIR // NSUB  # 8 pairs per sub-block
GROUP = 4          # pairs per psd PSUM group
EXLO, EXHI = 16, 256  # window-local col range exported as colsum partials
EXW = EXHI - EXLO     # 240
NF16 = 6           # tiles 0..5 -> fp16 DVE min path; 6 Pool fp8 min; 7 ACT fp8 abs
WSCALE = 64.0      # host premultiplies W by this; evac rescales by 1/WSCALE

_CACHE = {}


def _build_program(reps=1):
    import concourse.bass as bass
    import concourse.tile as tile
    from concourse import mybir
    from contextlib import ExitStack

    f32 = mybir.dt.float32
    f16 = mybir.dt.float16
    f8 = mybir.dt.float8e4
    Alu = mybir.AluOpType
    Act = mybir.ActivationFunctionType
    PM = mybir.MatmulPerfMode

    nc = bass.Bass()
    xTr_d = nc.declare_dram_parameter("xTr8", [NIN, TILEW], f8, isOutput=False)
    w8_d = nc.declare_dram_parameter("w8", [8 * 4 * 128, 256], f8, isOutput=False)
    b_d = nc.declare_dram_parameter("b", [FOUT], f32, isOutput=False)
    ind16_d = nc.declare_dram_parameter("ind16", [NF16 * 128, NBF], f16, isOutput=False)
    ind8dr_d = nc.declare_dram_parameter("ind8dr", [128, 2 * NBF], f8, isOutput=False)
    ind8dr1_d = nc.declare_dram_parameter("ind8dr1", [128, 2 * 128], f8, isOutput=False)
    ind8s_d = nc.declare_dram_parameter("ind8s", [128, NBF], f8, isOutput=False)
    ones16_d = nc.declare_dram_parameter("ones16", [128, NBF], f16, isOutput=False)
    repl64_d = nc.declare_dram_parameter("repl64", [NBF, 128], f16, isOutput=False)
    o_d = nc.declare_dram_parameter("o", [128, NPAIR], f32, isOutput=True)
    cp_d = nc.declare_dram_parameter("cpart", [128, 2 * EXW], f32, isOutput=True)

    with tile.TileContext(nc) as tc, ExitStack() as ctx:
        singles = ctx.enter_context(tc.tile_pool(name="singles", bufs=1))
        scr16 = ctx.enter_context(tc.tile_pool(name="scr16", bufs=36))
        scr8 = ctx.enter_context(tc.tile_pool(name="scr8", bufs=10))
        epool = ctx.enter_context(tc.tile_pool(name="epool", bufs=8))
        psA = ctx.enter_context(tc.tile_pool(name="psA", bufs=2, space="PSUM"))
        psQ = ctx.enter_context(tc.tile_pool(name="psQ", bufs=1, space="PSUM"))
        psC = ctx.enter_context(tc.tile_pool(name="psC", bufs=1, space="PSUM"))
        psB = ctx.enter_context(tc.tile_pool(name="psB", bufs=GROUP, space="PSUM"))

        dma = nc.default_dma_engine

        # ---- persistent loads -------------------------------------------
        # t=0's W slabs and the x tiles go first so the projection can
        # start before the whole W stream lands
        wt8 = [[None] * 4 for _ in range(8)]
        xr8 = []

        def w_tile(t, kb2):
            tl = singles.tile([128, 2, 128], f8, name=f"w{t}_{kb2}",
                              tag=f"w{t}_{kb2}")
            r0 = (t * 4 + kb2) * 128
            dma.dma_start(out=tl, in_=w8_d[r0:r0 + 128, :].rearrange(
                "p (two c) -> p two c", two=2))
            wt8[t][kb2] = tl

        for kb2 in range(4):
            w_tile(0, kb2)
        for kb2 in range(4):
            tl = singles.tile([128, 2, TILEW], f8, name=f"xr{kb2}",
                              tag=f"xr{kb2}")
            dma.dma_start(out=tl, in_=xTr_d[256 * kb2:256 * (kb2 + 1), :]
                          .rearrange("(two p) j -> p two j", two=2))
            xr8.append(tl)
        for t in range(1, 8):
            for kb2 in range(4):
                w_tile(t, kb2)

        b_sb = singles.tile([128, 8], f32, name="b_sb", tag="b_sb")
        dma.dma_start(out=b_sb, in_=b_d.rearrange("(t p) -> p t", p=128))
        ind16 = [singles.tile([128, NBF], f16, name=f"ind{t}", tag=f"ind{t}")
                 for t in range(NF16)]
        for t in range(NF16):
            dma.dma_start(out=ind16[t], in_=ind16_d[128 * t:128 * (t + 1), :])
        ind8dr = singles.tile([128, 2, NBF], f8, name="ind8dr", tag="ind8dr")
        dma.dma_start(out=ind8dr, in_=ind8dr_d.rearrange(
            "p (two c) -> p two c", two=2))
        # h=1 variant: full-width lhsT, zero low half -- DoubleRow matmuls
        # must target PSUM partition 0, so the odd row writes [0:128) with
        # +0 accumulated into the even row's half
        ind8dr1 = singles.tile([128, 2, 128], f8, name="ind8dr1", tag="ind8dr1")
        dma.dma_start(out=ind8dr1, in_=ind8dr1_d.rearrange(
            "p (two c) -> p two c", two=2))
        ind8s = singles.tile([128, NBF], f8, name="ind8s", tag="ind8s")
        dma.dma_start(out=ind8s, in_=ind8s_d[:, :])
        ones16 = singles.tile([128, NBF], f16, name="ones16", tag="ones16")
        dma.dma_start(out=ones16, in_=ones16_d[:, :])
        repl64 = singles.tile([NBF, 128], f16, name="repl64", tag="repl64")
        dma.dma_start(out=repl64, in_=repl64_d[:, :])

        # persistent compute tiles (rewritten each rep)
        mt16 = [singles.tile([128, TILEW], f16, name=f"mt{t}", tag=f"mt{t}")
                for t in range(NF16)]
        mt8 = [singles.tile([128, TILEW], f8, name=f"mt8_{t}", tag=f"mt8_{t}")
               for t in (6, 7)]
        mc32 = [singles.tile([128, IB], f32, name=f"mc{t}", tag=f"mc{t}")
                for t in range(NF16)]
        mc8v = [singles.tile([128, IB], f32, name=f"mc8_{t}", tag=f"mc8_{t}")
                for t in (6, 7)]
        S16 = singles.tile([NBF, TILEW], f16, name="S16", tag="S16")
        negS2 = singles.tile([128, NPAIR], f32, name="negS2", tag="negS2")
        oacc = singles.tile([128, NPAIR], f32, name="oacc", tag="oacc")
        cp_sb = singles.tile([128, 2 * EXW], f32, name="cp", tag="cp")

        def one_rep():
            # ---- mT = W @ x^T (fp8 DoubleRow), evac (+b, /WSCALE) -------
            for t in range(8):
                ps = psA.tile([128, TILEW], f32, name="mps", tag="mps")
                for kb2 in range(4):
                    nc.tensor.matmul(
                        ps, lhsT=wt8[t][kb2], rhs=xr8[kb2],
                        start=(kb2 == 0), stop=(kb2 == 3),
                        perf_mode=PM.DoubleRow,
                    )
                if t < NF16:
                    nc.vector.tensor_scalar(
                        out=mt16[t], in0=ps, scalar1=1.0 / WSCALE,
                        op0=Alu.mult, scalar2=b_sb[:, t:t + 1], op1=Alu.add)
                    nc.scalar.activation(
                        out=mc32[t], in_=ps[:, 0:IB], func=Act.Identity,
                        bias=b_sb[:, t:t + 1], scale=1.0 / WSCALE)
                else:
                    k = t - NF16
                    nc.scalar.activation(
                        out=mt8[k], in_=ps, func=Act.Identity,
                        bias=b_sb[:, t:t + 1], scale=1.0 / WSCALE)
                    nc.scalar.activation(
                        out=mc8v[k], in_=mt8[k][:, 0:IB], func=Act.Copy,
                        bias=0.0, scale=1.0)

            # ---- psq = 2*S over min tiles (0..6); S16, negS2 ------------
            psq = psQ.tile([128, TILEW], f32, name="psq", tag="psq")
            for t in range(NF16):
                nc.tensor.matmul(
                    psq[0:NBF, :], lhsT=ind16[t], rhs=mt16[t],
                    start=(t == 0), stop=False)
            nc.tensor.matmul(
                psq[0:NBF, :], lhsT=ind8s, rhs=mt8[0],
                start=False, stop=True)
            psq_pairs = psq[0:NBF, 0:IB].rearrange("b (p two) -> b two p", two=2)
            nc.scalar.activation(
                out=negS2[0:NBF, :], in_=psq_pairs[:, 0, :],
                func=Act.Copy, bias=0.0, scale=-0.5)
            nc.scalar.activation(
                out=negS2[NBF:128, :], in_=psq_pairs[:, 1, :],
                func=Act.Copy, bias=0.0, scale=-0.5)
            nc.scalar.activation(
                out=S16, in_=psq[0:NBF, :], func=Act.Copy, bias=0.0, scale=1.0)

            # colsum accumulators: 4 partition-disjoint PSUM regions.
            # caccA holds sub-blocks 0 (p0:64) / 2 (p64:128); caccB (reusing
            # psq's bank, dead after S16/negS2) holds 1 / 3.
            caccA = psC.tile([128, EXW], f32, name="caccA", tag="caccA")
            caccB = psQ.tile([128, EXW], f32, name="caccB", tag="psq")

            def cacc_ap(s):
                t_ = caccA if s % 2 == 0 else caccB
                lo = 64 * (s // 2)
                return t_[lo:lo + NBF, :]

            # ---- pairwise loop: tile-outer over groups of GROUP pairs ---
            for g0 in range(0, NPAIR, GROUP):
                pairs = list(range(g0, g0 + GROUP))
                soff = 16 * (g0 // PPS)  # window offset, same for the group
                psd = {}
                for p in pairs:
                    psd[p] = psB.tile([128, WIN], f32, name="psd", tag="psd")

                # fp8 producers first (slow engines), DVE fp16 after
                mn8t = {}
                for p in pairs:
                    for h in range(2):
                        i = 2 * p + h
                        m8 = scr8.tile([128, 2, WIN], f8, name="mn8", tag="mn8")
                        nc.gpsimd.tensor_scalar_min(
                            m8[:, 0, :], mt8[0][:, soff:soff + WIN],
                            mc8v[0][:, i:i + 1])
                        nc.scalar.activation(
                            out=m8[:, 1, :], in_=mt8[1][:, soff:soff + WIN],
                            func=Act.Abs, bias=mc8v[1][:, i:i + 1], scale=-1.0)
                        mn8t[(p, h)] = m8

                for e in range(NF16):
                    for h in range(2):
                        for p in pairs:
                            i = 2 * p + h
                            mn = scr16.tile([128, WIN], f16, name="mn", tag="mn")
                            nc.vector.tensor_scalar_min(
                                mn, mt16[e][:, soff:soff + WIN],
                                mc32[e][:, i:i + 1])
                            nc.tensor.matmul(
                                psd[p][NBF * h:NBF * (h + 1), :],
                                lhsT=ind16[e], rhs=mn,
                                start=(e == 0), stop=False)
                for p in pairs:
                    nc.tensor.matmul(
                        psd[p][0:NBF, :], lhsT=ind8dr, rhs=mn8t[(p, 0)],
                        start=False, stop=False, perf_mode=PM.DoubleRow)
                for p in pairs:
                    nc.tensor.matmul(
                        psd[p][:, :], lhsT=ind8dr1, rhs=mn8t[(p, 1)],
                        start=False, stop=False, perf_mode=PM.DoubleRow,
                        skip_group_check=True)
                for p in pairs:
                    nc.tensor.matmul(
                        psd[p][:, :], lhsT=repl64,
                        rhs=S16[:, soff:soff + WIN],
                        start=False, stop=True, skip_group_check=True)

                s = g0 // PPS
                for p in pairs:
                    E = epool.tile([128, WIN], f16, name="E", tag="E")
                    nc.scalar.activation(
                        out=E, in_=psd[p], func=Act.Exp,
                        bias=negS2[:, p:p + 1], scale=1.0,
                        accum_out=oacc[:, p:p + 1])
                    nc.tensor.matmul(
                        cacc_ap(s), lhsT=ones16, rhs=E[:, EXLO:EXHI],
                        start=(p % PPS == 0), stop=(p % PPS == PPS - 1),
                        skip_group_check=True)

            dma.dma_start(out=o_d[:, :], in_=oacc)
            nc.scalar.activation(out=cp_sb[:, 0:EXW], in_=caccA,
                                 func=Act.Copy, bias=0.0, scale=1.0)
            nc.scalar.activation(out=cp_sb[:, EXW:2 * EXW], in_=caccB,
                                 func=Act.Copy, bias=0.0, scale=1.0)
            dma.dma_start(out=cp_d[:, :], in_=cp_sb)

        for _ in range(reps):
            one_rep()

    _dedup_ldweights(nc)
    _split_multi_waits(nc, mybir)
    return nc


def _dedup_ldweights(nc):
    """Drop PE InstLdweights whose weights AP + mode matches the previous
    load and which carry no sync (weights stay resident in the PE array).
    Self-loading matmuls (fp32/fp32r) clobber resident weights and
    invalidate the tracking."""
    f = nc.m.functions[0]
    n = 0
    for blk in f.blocks:
        last_key = None
        keep = []
        for inst in blk.instructions:
            if str(inst.engine) == "EngineType.PE":
                tname = type(inst).__name__
                if tname == "InstLdweights":
                    si = inst.sync_info
                    clean = not si or (not si.on_wait and not si.on_update)
                    key = (
                        str(inst.ins[0]),
                        str(inst.perf_mode),
                        str(inst.is_transpose),
                        str(inst.tile_position),
                    )
                    if clean and key == last_key:
                        n += 1
                        continue
                    last_key = key
                elif tname == "InstMatmult":
                    if inst.ldweights is not False:
                        last_key = None
            keep.append(inst)
        blk.instructions[:] = keep
    return n


def _split_multi_waits(nc, mybir):
    """Walrus here rejects >1 sync wait per instruction; hoist extras onto
    single-wait NoOps just before, on the same engine queue."""
    f = nc.m.functions[0]
    n_split = 0
    for blk in f.blocks:
        idx = 0
        while idx < len(blk.instructions):
            inst = blk.instructions[idx]
            si = inst.sync_info
            waits = list(si.on_wait) if si is not None and si.on_wait else []
            if len(waits) > 1:
                bysem = {}
                for w in waits:
                    k = w.id
                    if k not in bysem or (w.wait_value or 0) > (
                        bysem[k].wait_value or 0
                    ):
                        bysem[k] = w
                waits = list(bysem.values())
                for w in waits[:-1]:
                    nop = mybir.InstNoOp(
                        name=nc.get_next_instruction_name(), ins=[], outs=[]
                    )
                    nop.engine = inst.engine
                    nop.sync_info = mybir.SyncInfo(on_wait=[w], on_update=[])
                    blk.instructions.insert(idx, nop)
                    idx += 1
                    n_split += 1
                si.on_wait = [waits[-1]]
            idx += 1
    return n_split


def _get_program(reps=1):
    key = f"nc{reps}"
    if key not in _CACHE:
        _CACHE[key] = _build_program(reps)
    return _CACHE[key]


def _make_indicators():
    import ml_dtypes
    f8 = ml_dtypes.float8_e4m3fn
    r = np.arange(NF16 * 128)
    ind16 = np.zeros((NF16 * 128, NBF), dtype=np.float16)
    ind16[r, (r // 128) * 8 + (r % 128) // NCD] = 2.0
    p = np.arange(128)
    ind8dr = np.zeros((128, 2 * NBF), dtype=f8)
    ind8dr[p, 48 + p // NCD] = f8(2.0)            # half 0: tile 6 (min)
    ind8dr[p, NBF + 56 + p // NCD] = f8(-1.0)     # half 1: tile 7 (abs)
    ind8dr1 = np.zeros((128, 2 * 128), dtype=f8)  # cols 0:64 zero (even row)
    ind8dr1[p, NBF + 48 + p // NCD] = f8(2.0)
    ind8dr1[p, 128 + NBF + 56 + p // NCD] = f8(-1.0)
    ind8s = np.zeros((128, NBF), dtype=f8)
    ind8s[p, 48 + p // NCD] = f8(2.0)
    ones16 = np.zeros((128, NBF), dtype=np.float16)
    bb = np.arange(NBF)
    ones16[bb, bb] = 1.0
    ones16[NBF + bb, bb] = 1.0
    repl64 = np.zeros((NBF, 128), dtype=np.float16)
    repl64[bb, bb] = -0.5
    repl64[bb, NBF + bb] = -0.5
    return ind16, ind8dr, ind8dr1, ind8s, ones16, repl64


def make_in_maps(x, W, b):
    import ml_dtypes
    f8 = ml_dtypes.float8_e4m3fn
    x = np.ascontiguousarray(x, dtype=np.float32)
    W = np.ascontiguousarray(W, dtype=np.float32)
    b = np.ascontiguousarray(b, dtype=np.float32)
    ind16, ind8dr, ind8dr1, ind8s, ones16, repl64 = _make_indicators()

    # W layout: blocks of (t, kb2): [128 p, 2 i, 128 col] from
    # wTr8[256*kb2 + 128*i + p, 128*t + col]; wTr8 = fp8(64 * W.T)
    wTr8 = (W.T * WSCALE).astype(f8)
    w8 = np.empty((8 * 4 * 128, 256), dtype=f8)
    for t in range(8):
        for kb2 in range(4):
            blk = w8[(t * 4 + kb2) * 128:(t * 4 + kb2 + 1) * 128]
            for i in range(2):
                blk[:, i * 128:(i + 1) * 128] = (
                    wTr8[256 * kb2 + 128 * i:256 * kb2 + 128 * (i + 1),
                         128 * t:128 * (t + 1)])

    in_maps = []
    for c in range(NCORES):
        xr = np.roll(x, -IB * c, axis=0)
        xTr8 = np.ascontiguousarray(xr.T[:, :TILEW]).astype(f8)
        in_maps.append({
            "xTr8": xTr8, "w8": w8, "b": b, "ind16": ind16,
            "ind8dr": ind8dr, "ind8dr1": ind8dr1, "ind8s": ind8s,
            "ones16": ones16, "repl64": repl64,
        })
    return in_maps


def kernel(x, W, b):
    from concourse.bass_utils import run_bass_kernel_spmd

    x = np.ascontiguousarray(x, dtype=np.float32)
    nc = _get_program()
    in_maps = make_in_maps(x, W, b)

    res = run_bass_kernel_spmd(nc, in_maps, list(range(NCORES)), trace=False)
    _CACHE["last_results"] = res

    o_full = np.zeros((NB, NBF), dtype=np.float64)
    for c in range(NCORES):
        oc = np.asarray(res.results[c]["o"], dtype=np.float64)
        o_core = np.empty((IB, NBF))
        o_core[0::2, :] = oc[0:NBF, :].T
        o_core[1::2, :] = oc[NBF:128, :].T
        o_full[IB * c:IB * (c + 1), :] += o_core
        cp = np.asarray(res.results[c]["cpart"], dtype=np.float64)
        for s in range(NSUB):
            blk = cp[64 * (s // 2):64 * (s // 2) + NBF,
                     EXW * (s % 2):EXW * (s % 2 + 1)]
            rows = (IB * c + SUBW * s + EXLO + np.arange(EXW)) % NB
            o_full[rows, :] += blk.T
    return np.concatenate([x, o_full.astype(np.float32)], axis=1)


# revision 11
# speedup vs baseline: 1.4172x; 1.3372x over previous
"""Trainium2 Bass kernel for MinibatchDiscrimination (v3).

Reference computation (fp32):
    m = (x @ W.T + b).reshape(nb, 64, 16)            # nb=512
    d[i,j,B] = sum_c |m[i,B,c] - m[j,B,c]|
    o[i,B]   = sum_j exp(-d[i,j,B])
    out      = concat(x, o, axis=1)                   # (512, 1088)

Strategy (8 cores, 32-ring symmetric decomposition): core c owns output
rows R_c = [64c, 64c+64), x row-rotated per core.  Rows are split into
4 sub-blocks of 16; the 512 columns form a ring of 32 16-blocks.  A row
in 16-block r computes exp(-d) over the 272-column window of blocks
r..r+16 (local cols [16s, 16s+272) for sub-block s).  Row sums cover
distances 0..+16; per-column sums over window cols [16,256) (distances
+1..+15) are exported and added host-side to the owners of those rows
(by symmetry of d).  Distance-16 blocks are computed by both endpoint
blocks, each feeding only its own row sums -- no double count.

On-device pipeline per core:
  mT = W @ x^T + b as tiles [128 (B,c), 320 j]: fp8 DoubleRow matmuls
      (x, 64*W in fp8e4m3; PSUM scaled by 1/64 on evacuation).  Tiles
      0..5 evacuate to fp16 (DVE), tiles 6..7 to fp8 (ACT) for the
      fp8 pairwise path.
  Per row i: tiles 0..5 min-path on DVE (fp16, 4x mode); tile 6
      min-path on Pool (fp8); tile 7 abs-path on ACT (fp8).  The two
      fp8 results land in the halves of one [128,2,272] tile consumed
      by a single DoubleRow matmul (0.5 cyc/row); fp16 tiles use 6
      plain matmuls.  A per-pair replicate matmul folds -S_j into the
      same PSUM accumulation, so ACT's Exp (bias=-S_i) directly yields
      exp(-d) in fp16 with accum_out giving the row sums for free.
  colsum: ones16^T @ E[:,16:256] accumulated per sub-block into
      partition-disjoint PSUM regions; evacuated once.

Host assembles: out = concat(x, rowsums + scattered colsums, axis=1).

reps>1 unrolls the body for marginal (steady-state) timing.
The container's walrus rejects >1 sync wait per instruction; the
_split_multi_waits pass legalizes (hoists extras onto NoOps)."""

import os
import sys
import numpy as np

if "/opt/trn_rl_repo" not in sys.path:
    sys.path.insert(0, "/opt/trn_rl_repo")

NB = 512          # batch rows
NIN = 1024        # n_in
NBF = 64          # n_B
NCD = 16          # n_C
FOUT = NBF * NCD  # 1024 projection features
NCORES = 8
IB = NB // NCORES  # 64 output rows per core
TILEW = 320        # mt tile width (union of sub-block windows)
WIN = 272          # per-row j-window (17 x 16-blocks)
SUBW = 16          # sub-block row granularity
NSUB = IB // SUBW  # 4 sub-blocks per core
NPAIR = IB // 2    # 32 psd pair tiles
PPS = NPAIR // NSUB  # 8 pairs per sub-block
GROUP = 3          # pairs per psd PSUM group
EXLO, EXHI = 16, 256  # window-local col range exported as colsum partials
EXW = EXHI - EXLO     # 240
NF16 = 6           # tiles 0..5 -> fp16 DVE min path; 6 Pool fp8 min; 7 ACT fp8 abs
WSCALE = 64.0      # host premultiplies W by this; evac rescales by 1/WSCALE

_CACHE = {}


def _build_program(reps=1):
    import concourse.bass as bass
    import concourse.tile as tile
    from concourse import mybir
    from contextlib import ExitStack

    f32 = mybir.dt.float32
    f16 = mybir.dt.float16
    f8 = mybir.dt.float8e4
    Alu = mybir.AluOpType
    Act = mybir.ActivationFunctionType
    PM = mybir.MatmulPerfMode

    nc = bass.Bass()
    xTr_d = nc.declare_dram_parameter("xTr8", [NIN, TILEW], f8, isOutput=False)
    w8_d = nc.declare_dram_parameter("w8", [NIN, FOUT], f8, isOutput=False)
    b_d = nc.declare_dram_parameter("b", [FOUT], f32, isOutput=False)
    ind16_d = nc.declare_dram_parameter("ind16", [128, (NF16 + 1) * NBF], f16, isOutput=False)
    ind8dr_d = nc.declare_dram_parameter("ind8dr", [128, 2 * NBF + 2 * 128 + NBF], f8, isOutput=False)
    repl64_d = nc.declare_dram_parameter("repl64", [NBF, 128], f16, isOutput=False)
    o_d = nc.declare_dram_parameter("o", [128, NPAIR], f32, isOutput=True)
    cp_d = nc.declare_dram_parameter("cpart", [128, 2 * EXW], f32, isOutput=True)

    with tile.TileContext(nc) as tc, ExitStack() as ctx:
        singles = ctx.enter_context(tc.tile_pool(name="singles", bufs=1))
        scr16 = ctx.enter_context(tc.tile_pool(name="scr16", bufs=36))
        scr8 = ctx.enter_context(tc.tile_pool(name="scr8", bufs=10))
        epool = ctx.enter_context(tc.tile_pool(name="epool", bufs=8))
        psA = ctx.enter_context(tc.tile_pool(name="psA", bufs=2, space="PSUM"))
        psQ = ctx.enter_context(tc.tile_pool(name="psQ", bufs=1, space="PSUM"))
        psC = ctx.enter_context(tc.tile_pool(name="psC", bufs=1, space="PSUM"))
        psB = ctx.enter_context(tc.tile_pool(name="psB", bufs=GROUP + 1, space="PSUM"))

        dma = nc.default_dma_engine

        # ---- persistent loads -------------------------------------------
        # W in 4 big contiguous DMAs (k-major rows; per-partition lines are
        # two 1KB chunks), slab 0 and the x tiles first so the projection
        # starts before the whole stream lands
        wsl8 = []
        xr8 = []
        for kb2 in range(4):
            tl = singles.tile([128, 2, FOUT], f8, name=f"wsl{kb2}",
                              tag=f"wsl{kb2}")
            dma.dma_start(out=tl, in_=w8_d[256 * kb2:256 * (kb2 + 1), :]
                          .rearrange("(two p) c -> p two c", two=2))
            wsl8.append(tl)
            if kb2 == 0:
                for k2 in range(4):
                    tx = singles.tile([128, 2, TILEW], f8, name=f"xr{k2}",
                                      tag=f"xr{k2}")
                    dma.dma_start(out=tx, in_=xTr_d[256 * k2:256 * (k2 + 1), :]
                                  .rearrange("(two p) j -> p two j", two=2))
                    xr8.append(tx)

        b_sb = singles.tile([128, 8], f32, name="b_sb", tag="b_sb")
        dma.dma_start(out=b_sb, in_=b_d.rearrange("(t p) -> p t", p=128))
        # f16 constants in one DMA: ind16 tiles (partition-major) + ones16
        c16 = singles.tile([128, (NF16 + 1) * NBF], f16, name="c16", tag="c16")
        dma.dma_start(out=c16, in_=ind16_d[:, :])
        ind16 = [c16[:, NBF * t:NBF * (t + 1)] for t in range(NF16)]
        ones16 = c16[:, NBF * NF16:NBF * (NF16 + 1)]
        # fp8 constants in one DMA: ind8dr (2x64) + ind8dr1 (2x128) + ind8s
        c8 = singles.tile([128, 2 * NBF + 2 * 128 + NBF], f8, name="c8", tag="c8")
        dma.dma_start(out=c8, in_=ind8dr_d[:, :])
        ind8dr = c8[:, 0:2 * NBF].rearrange("p (two c) -> p two c", two=2)
        ind8dr1 = c8[:, 2 * NBF:2 * NBF + 256].rearrange(
            "p (two c) -> p two c", two=2)
        ind8s = c8[:, 2 * NBF + 256:2 * NBF + 256 + NBF]
        repl64 = singles.tile([NBF, 128], f16, name="repl64", tag="repl64")
        dma.dma_start(out=repl64, in_=repl64_d[:, :])

        # persistent compute tiles (rewritten each rep)
        mt16 = [singles.tile([128, TILEW], f16, name=f"mt{t}", tag=f"mt{t}")
                for t in range(NF16)]
        mt8 = [singles.tile([128, TILEW], f8, name=f"mt8_{t}", tag=f"mt8_{t}")
               for t in (6, 7)]
        mc32 = [singles.tile([128, IB], f32, name=f"mc{t}", tag=f"mc{t}")
                for t in range(NF16)]
        mc8v = [singles.tile([128, IB], f32, name=f"mc8_{t}", tag=f"mc8_{t}")
                for t in (6, 7)]
        S16 = singles.tile([NBF, TILEW], f16, name="S16", tag="S16")
        negS2 = singles.tile([128, NPAIR], f32, name="negS2", tag="negS2")
        oacc = singles.tile([128, NPAIR], f32, name="oacc", tag="oacc")
        cp_sb = singles.tile([128, 2 * EXW], f32, name="cp", tag="cp")

        def one_rep():
            # ---- mT = W @ x^T (fp8 DoubleRow), evac (+b, /WSCALE) -------
            for t in range(8):
                ps = psA.tile([128, TILEW], f32, name="mps", tag="mps")
                for kb2 in range(4):
                    nc.tensor.matmul(
                        ps, lhsT=wsl8[kb2][:, :, 128 * t:128 * (t + 1)],
                        rhs=xr8[kb2],
                        start=(kb2 == 0), stop=(kb2 == 3),
                        perf_mode=PM.DoubleRow,
                    )
                if t < NF16:
                    nc.vector.tensor_scalar(
                        out=mt16[t], in0=ps, scalar1=1.0 / WSCALE,
                        op0=Alu.mult, scalar2=b_sb[:, t:t + 1], op1=Alu.add)
                    nc.scalar.activation(
                        out=mc32[t], in_=ps[:, 0:IB], func=Act.Identity,
                        bias=b_sb[:, t:t + 1], scale=1.0 / WSCALE)
                else:
                    k = t - NF16
                    nc.scalar.activation(
                        out=mt8[k], in_=ps, func=Act.Identity,
                        bias=b_sb[:, t:t + 1], scale=1.0 / WSCALE)
                    nc.scalar.activation(
                        out=mc8v[k], in_=mt8[k][:, 0:IB], func=Act.Copy,
                        bias=0.0, scale=1.0)

            # ---- psq = 2*S over min tiles (0..6); S16, negS2 ------------
            psq = psQ.tile([128, TILEW], f32, name="psq", tag="psq")
            for t in range(NF16):
                nc.tensor.matmul(
                    psq[0:NBF, :], lhsT=ind16[t], rhs=mt16[t],
                    start=(t == 0), stop=False)
            nc.tensor.matmul(
                psq[0:NBF, :], lhsT=ind8s, rhs=mt8[0],
                start=False, stop=True)
            psq_pairs = psq[0:NBF, 0:IB].rearrange("b (p two) -> b two p", two=2)
            nc.scalar.activation(
                out=negS2[0:NBF, :], in_=psq_pairs[:, 0, :],
                func=Act.Copy, bias=0.0, scale=-0.5)
            nc.scalar.activation(
                out=negS2[NBF:128, :], in_=psq_pairs[:, 1, :],
                func=Act.Copy, bias=0.0, scale=-0.5)
            nc.scalar.activation(
                out=S16, in_=psq[0:NBF, :], func=Act.Copy, bias=0.0, scale=1.0)

            # colsum accumulators: 4 partition-disjoint PSUM regions.
            # caccA holds sub-blocks 0 (p0:64) / 2 (p64:128); caccB (reusing
            # psq's bank, dead after S16/negS2) holds 1 / 3.
            caccA = psC.tile([128, EXW], f32, name="caccA", tag="caccA")
            caccB = psQ.tile([128, EXW], f32, name="caccB", tag="psq")

            def cacc_ap(s):
                t_ = caccA if s % 2 == 0 else caccB
                lo = 64 * (s // 2)
                return t_[lo:lo + NBF, :]

            # ---- pairwise loop: tile-outer over groups of GROUP pairs ---
            group_list = []
            for s_ in range(NSUB):
                base = PPS * s_
                group_list += [[base, base + 1, base + 2],
                               [base + 3, base + 4, base + 5],
                               [base + 6, base + 7]]
            for pairs in group_list:
                soff = 16 * (pairs[0] // PPS)  # window offset, same per group
                psd = {}
                for p in pairs:
                    psd[p] = psB.tile([128, WIN], f32, name="psd", tag="psd")

                # fp8 producers first (slow engines), DVE fp16 after
                mn8t = {}
                for p in pairs:
                    for h in range(2):
                        i = 2 * p + h
                        m8 = scr8.tile([128, 2, WIN], f8, name="mn8", tag="mn8")
                        nc.gpsimd.tensor_scalar_min(
                            m8[:, 0, :], mt8[0][:, soff:soff + WIN],
                            mc8v[0][:, i:i + 1])
                        nc.scalar.activation(
                            out=m8[:, 1, :], in_=mt8[1][:, soff:soff + WIN],
                            func=Act.Abs, bias=mc8v[1][:, i:i + 1], scale=-1.0)
                        mn8t[(p, h)] = m8

                for e in range(NF16):
                    for h in range(2):
                        for p in pairs:
                            i = 2 * p + h
                            mn = scr16.tile([128, WIN], f16, name="mn", tag="mn")
                            nc.vector.tensor_scalar_min(
                                mn, mt16[e][:, soff:soff + WIN],
                                mc32[e][:, i:i + 1])
                            nc.tensor.matmul(
                                psd[p][NBF * h:NBF * (h + 1), :],
                                lhsT=ind16[e], rhs=mn,
                                start=(e == 0), stop=False)
                for p in pairs:
                    nc.tensor.matmul(
                        psd[p][0:NBF, :], lhsT=ind8dr, rhs=mn8t[(p, 0)],
                        start=False, stop=False, perf_mode=PM.DoubleRow)
                for p in pairs:
                    nc.tensor.matmul(
                        psd[p][:, :], lhsT=ind8dr1, rhs=mn8t[(p, 1)],
                        start=False, stop=False, perf_mode=PM.DoubleRow,
                        skip_group_check=True)
                for p in pairs:
                    nc.tensor.matmul(
                        psd[p][:, :], lhsT=repl64,
                        rhs=S16[:, soff:soff + WIN],
                        start=False, stop=True, skip_group_check=True)

                s = pairs[0] // PPS
                for p in pairs:
                    E = epool.tile([128, WIN], f16, name="E", tag="E")
                    nc.scalar.activation(
                        out=E, in_=psd[p], func=Act.Exp,
                        bias=negS2[:, p:p + 1], scale=1.0,
                        accum_out=oacc[:, p:p + 1])
                    nc.tensor.matmul(
                        cacc_ap(s), lhsT=ones16, rhs=E[:, EXLO:EXHI],
                        start=(p % PPS == 0), stop=(p % PPS == PPS - 1),
                        skip_group_check=True)

            dma.dma_start(out=o_d[:, :], in_=oacc)
            nc.scalar.activation(out=cp_sb[:, 0:EXW], in_=caccA,
                                 func=Act.Copy, bias=0.0, scale=1.0)
            nc.scalar.activation(out=cp_sb[:, EXW:2 * EXW], in_=caccB,
                                 func=Act.Copy, bias=0.0, scale=1.0)
            dma.dma_start(out=cp_d[:, :], in_=cp_sb)

        for _ in range(reps):
            one_rep()

    _dedup_ldweights(nc)
    _split_multi_waits(nc, mybir)
    return nc


def _dedup_ldweights(nc):
    """Drop PE InstLdweights whose weights AP + mode matches the previous
    load and which carry no sync (weights stay resident in the PE array).
    Self-loading matmuls (fp32/fp32r) clobber resident weights and
    invalidate the tracking."""
    f = nc.m.functions[0]
    n = 0
    for blk in f.blocks:
        last_key = None
        keep = []
        for inst in blk.instructions:
            if str(inst.engine) == "EngineType.PE":
                tname = type(inst).__name__
                if tname == "InstLdweights":
                    si = inst.sync_info
                    clean = not si or (not si.on_wait and not si.on_update)
                    key = (
                        str(inst.ins[0]),
                        str(inst.perf_mode),
                        str(inst.is_transpose),
                        str(inst.tile_position),
                    )
                    if clean and key == last_key:
                        n += 1
                        continue
                    last_key = key
                elif tname == "InstMatmult":
                    if inst.ldweights is not False:
                        last_key = None
            keep.append(inst)
        blk.instructions[:] = keep
    return n


def _split_multi_waits(nc, mybir):
    """Walrus here rejects >1 sync wait per instruction; hoist extras onto
    single-wait NoOps just before, on the same engine queue."""
    f = nc.m.functions[0]
    n_split = 0
    for blk in f.blocks:
        idx = 0
        while idx < len(blk.instructions):
            inst = blk.instructions[idx]
            si = inst.sync_info
            waits = list(si.on_wait) if si is not None and si.on_wait else []
            if len(waits) > 1:
                bysem = {}
                for w in waits:
                    k = w.id
                    if k not in bysem or (w.wait_value or 0) > (
                        bysem[k].wait_value or 0
                    ):
                        bysem[k] = w
                waits = list(bysem.values())
                for w in waits[:-1]:
                    nop = mybir.InstNoOp(
                        name=nc.get_next_instruction_name(), ins=[], outs=[]
                    )
                    nop.engine = inst.engine
                    nop.sync_info = mybir.SyncInfo(on_wait=[w], on_update=[])
                    blk.instructions.insert(idx, nop)
                    idx += 1
                    n_split += 1
                si.on_wait = [waits[-1]]
            idx += 1
    return n_split


def _get_program(reps=1):
    key = f"nc{reps}"
    if key not in _CACHE:
        _CACHE[key] = _build_program(reps)
    return _CACHE[key]


def _make_indicators():
    import ml_dtypes
    f8 = ml_dtypes.float8_e4m3fn
    p0 = np.arange(128)
    ind16 = np.zeros((128, (NF16 + 1) * NBF), dtype=np.float16)
    for t in range(NF16):
        ind16[p0, NBF * t + t * 8 + p0 // NCD] = 2.0
    p = np.arange(128)
    ind8 = np.zeros((128, 2 * NBF + 2 * 128 + NBF), dtype=f8)
    ind8[p, 48 + p // NCD] = f8(2.0)              # ind8dr half 0: tile 6 (min)
    ind8[p, NBF + 56 + p // NCD] = f8(-1.0)       # ind8dr half 1: tile 7 (abs)
    q = 2 * NBF                                   # ind8dr1: cols 0:64 zero
    ind8[p, q + NBF + 48 + p // NCD] = f8(2.0)
    ind8[p, q + 128 + NBF + 56 + p // NCD] = f8(-1.0)
    ind8[p, q + 256 + 48 + p // NCD] = f8(2.0)    # ind8s
    bb = np.arange(NBF)
    ind16[bb, NBF * NF16 + bb] = 1.0       # ones16 block
    ind16[NBF + bb, NBF * NF16 + bb] = 1.0
    repl64 = np.zeros((NBF, 128), dtype=np.float16)
    repl64[bb, bb] = -0.5
    repl64[bb, NBF + bb] = -0.5
    return ind16, ind8, repl64


def make_in_maps(x, W, b):
    import ml_dtypes
    f8 = ml_dtypes.float8_e4m3fn
    x = np.ascontiguousarray(x, dtype=np.float32)
    W = np.ascontiguousarray(W, dtype=np.float32)
    b = np.ascontiguousarray(b, dtype=np.float32)
    ind16, ind8, repl64 = _make_indicators()

    w8 = (W.T * WSCALE).astype(f8)  # [NIN, FOUT], k-major

    in_maps = []
    for c in range(NCORES):
        xr = np.roll(x, -IB * c, axis=0)
        xTr8 = np.ascontiguousarray(xr.T[:, :TILEW]).astype(f8)
        in_maps.append({
            "xTr8": xTr8, "w8": w8, "b": b, "ind16": ind16,
            "ind8dr": ind8, "repl64": repl64,
        })
    return in_maps


def kernel(x, W, b):
    from concourse.bass_utils import run_bass_kernel_spmd

    x = np.ascontiguousarray(x, dtype=np.float32)
    nc = _get_program()
    in_maps = make_in_maps(x, W, b)

    res = run_bass_kernel_spmd(nc, in_maps, list(range(NCORES)), trace=False)
    _CACHE["last_results"] = res

    o_full = np.zeros((NB, NBF), dtype=np.float64)
    for c in range(NCORES):
        oc = np.asarray(res.results[c]["o"], dtype=np.float64)
        o_core = np.empty((IB, NBF))
        o_core[0::2, :] = oc[0:NBF, :].T
        o_core[1::2, :] = oc[NBF:128, :].T
        o_full[IB * c:IB * (c + 1), :] += o_core
        cp = np.asarray(res.results[c]["cpart"], dtype=np.float64)
        for s in range(NSUB):
            blk = cp[64 * (s // 2):64 * (s // 2) + NBF,
                     EXW * (s % 2):EXW * (s % 2 + 1)]
            rows = (IB * c + SUBW * s + EXLO + np.arange(EXW)) % NB
            o_full[rows, :] += blk.T
    return np.concatenate([x, o_full.astype(np.float32)], axis=1)
